# revision 7
# baseline (speedup 1.0000x reference)
"""Trainium2 Bass kernel for nn_DeconvBlock (offset conv -> deformable conv
-> BN+SiLU -> ConvTranspose2d(4,2,1) -> BN+SiLU), data-parallel over batch
on 8 NeuronCores with sync-BN allreduces.

v2: host-prepped layouts (token-major row-pair-duplicated gather table +
pre-padded channel-major image), one 2KB gather element per (pixel, tap),
symmetric x/y clip-selector math, FD=2048 combine ops, corner fold-add
absorbed into the PE accumulation, f16 output.
"""
import sys

sys.path.insert(0, "/opt/trn_rl_repo")

import numpy as np

import concourse.bass as bass
import concourse.mybir as mybir
from concourse import masks
from concourse.bacc import Bacc
from concourse.bass_types import AP
from concourse.bass_utils import run_bass_kernel_spmd
from concourse.tile import TileContext
from concourse.vector_clock import ScopedClock

# ---------------------------------------------------------------------------
# This walrus build rejects >1 sync-wait command on a Drain: split the
# TileContext tail-drain waits into a chain of single-wait drains.
from concourse import tile as _tile


def _patched_drain_and_barrier(self, tick_clock, wait_clock):
    nc = self.nc
    gc = tick_clock.global_clock
    procs = [(None, proc, tick) for proc, tick in enumerate(gc) if tick > 0]
    for scope, proc, tick in procs:
        sc = ScopedClock()
        sc.require_at_least(scope, proc, tick)
        d = nc.sync.drain()
        wait_clock.add_sem_waits(d.ins, sc)
    if not procs:
        nc.sync.drain()
    nc.all_engine_barrier()
    assert self.sems is not None
    popped = nc._tile_sem_poison_stack.pop()
    assert popped is self._sem_poison
    nc.clear_and_free_semaphores(list(self.sems.allocated().values()))
    nc.all_engine_barrier()


_tile.TileContext._drain_and_barrier = _patched_drain_and_barrier
# ---------------------------------------------------------------------------

F32 = mybir.dt.float32
F16 = mybir.dt.float16
I16 = mybir.dt.int16
I32 = mybir.dt.int32
ALU = mybir.AluOpType
ACTF = mybir.ActivationFunctionType

B, C, CO, H, W = 16, 256, 256, 32, 32
P = H * W  # 1024
NCORES = 8
BPC = B // NCORES  # batch elems per core
HO, WO = 2 * H, 2 * W
PO = HO * WO  # 4096
EPS = 1e-5
PADW = 34  # padded image row stride (34x34)
PPIX = PADW * PADW  # 1156

GP_ADD = False  # offload the corner-pair add to GpSimd


def apv(base, off, dims, nparts=None):
    """Free-dim view of an SBUF AP: keep its partition dim (stride), replace
    free dims. dims entries are [step, count] in elements of the tile row."""
    p = [base.ap[0][0], nparts if nparts is not None else base.ap[0][1]]
    return AP(tensor=base.tensor, offset=base.offset + off, ap=[p] + [list(d) for d in dims])


def dview(base, off, dims):
    """Arbitrary flat view of a DRAM AP."""
    return AP(tensor=base.tensor, offset=base.offset + off, ap=[list(d) for d in dims])


def _consts():
    q = np.arange(128)
    j = np.arange(8)
    k = np.arange(9)
    ky = k // 3 - 1
    kx = k % 3 - 1
    h = 4 * j[None, :, None] + (q[:, None, None] // 32)
    w = (q % 32)[:, None, None] + np.zeros((1, 8, 1))
    hky = (h + ky[None, None, :]) + np.zeros((128, 8, 9))
    wkx = (w + kx[None, None, :]) + np.zeros((128, 8, 9))
    e36 = np.zeros((36, 36 * 128), np.float16)
    for i in range(36):
        e36[i, i * 128 : (i + 1) * 128] = 1.0
    return hky.astype(np.float32), wkx.astype(np.float32), e36


def build_program(groups=None, bn_b=B):
    nc = Bacc()
    xt2_in = nc.declare_dram_parameter("xt2", [BPC * P + 1, 512], F16, isOutput=False).ap()
    xpad_in = nc.declare_dram_parameter("xpad", [BPC, 2, 128, PPIX], F16, isOutput=False).ap()
    woff_in = nc.declare_dram_parameter("w_off", [18, C * 9], F32, isOutput=False).ap()
    boff_in = nc.declare_dram_parameter("b_off", [18], F32, isOutput=False)
    wdef_in = nc.declare_dram_parameter("w_def", [CO, C * 9], F32, isOutput=False).ap()
    wdc_in = nc.declare_dram_parameter("w_dc", [C, CO * 16], F32, isOutput=False).ap()
    g1_in = nc.declare_dram_parameter("gamma1", [CO], F32, isOutput=False)
    b1_in = nc.declare_dram_parameter("beta1", [CO], F32, isOutput=False)
    g2_in = nc.declare_dram_parameter("gamma2", [CO], F32, isOutput=False)
    b2_in = nc.declare_dram_parameter("beta2", [CO], F32, isOutput=False)
    out_t = nc.declare_dram_parameter("out", [BPC, CO, PO], F16, isOutput=True).ap()

    hky_np, wkx_np, e36_np = _consts()
    hky_c = nc.inline_tensor(hky_np, "hky")
    wkx_c = nc.inline_tensor(wkx_np, "wkx")
    e36_c = nc.inline_tensor(e36_np, "e36")

    TT = nc.vector.tensor_tensor
    TS = nc.vector.tensor_scalar
    CP = nc.vector.tensor_copy

    with TileContext(nc) as tc:
        ex = tc.tile_pool
        with (
            ex(name="const", bufs=1) as cpool,
            ex(name="wts", bufs=1) as wpool,
            ex(name="work", bufs=1) as wk,
            ex(name="pipe", bufs=1) as pipe,
            ex(name="big", bufs=1) as big,
            ex(name="dram", bufs=1, space="DRAM") as dpool,
        ):
            # ---------------- constants ----------------
            id16 = cpool.tile([128, 128], F16)
            masks.make_identity(nc, id16[:])
            id32 = cpool.tile([128, 128], F32)
            masks.make_identity(nc, id32[:])
            hky = cpool.tile([128, 8, 9], F32)
            nc.sync.dma_start(hky[:], hky_c[:])
            wkx = cpool.tile([128, 8, 9], F32)
            nc.sync.dma_start(wkx[:], wkx_c[:])
            e36 = cpool.tile([36, 36 * 128], F16)
            nc.sync.dma_start(e36[:], e36_c[:])
            ones512 = cpool.tile([1, 512], F16)
            nc.vector.memset(ones512[:], 1.0)

            # ---------------- load + prep weights ----------------
            with ex(name="psW", bufs=2, space="PSUM") as psW, ex(name="prep", bufs=1) as prep:
                wof_raw = prep.tile([18, C * 9], F32, tag="wofr", name="wofr")
                nc.sync.dma_start(wof_raw[:], woff_in[:, :])
                wof16 = wpool.tile([18, C * 9], F16)
                CP(wof16[:], wof_raw[:])
                wot = wpool.tile([128, 2, 9, 18], F16)
                for cc in range(2):
                    for k in range(9):
                        tp = psW.tile([128, 18], F16, tag="tp18", name="tp18")
                        src = apv(wof16[:], cc * 1152 + k, [[9, 128]])
                        nc.tensor.transpose(tp[:], src, id16[0:18, 0:18])
                        CP(wot[:, cc, k, :], tp[:])
                bof32 = prep.tile([1, 18], F32, tag="bof32", name="bof32")
                nc.sync.dma_start(bof32[:], dview(boff_in.ap(), 0, [[18, 1], [1, 18]]))
                bof = wpool.tile([1, 18], F16)
                CP(bof[:], bof32[:])

                wdt = wpool.tile([128, 9, 2, 2, 128], F16)
                for co in range(2):
                    wdr = prep.tile([128, C * 9], F32, tag="wdr", name="wdr")
                    nc.sync.dma_start(wdr[:], wdef_in[co * 128 : (co + 1) * 128, :])
                    wdr16 = prep.tile([128, C * 9], F16, tag="wdr16", name="wdr16")
                    CP(wdr16[:], wdr[:])
                    for cc in range(2):
                        for k in range(9):
                            tp = psW.tile([128, 128], F16, tag="tp128", name="tp128")
                            src = apv(wdr16[:], cc * 1152 + k, [[9, 128]])
                            nc.tensor.transpose(tp[:], src, id16[:])
                            CP(wdt[:, k, cc, co, :], tp[:])

                wdc = wpool.tile([128, 2, CO * 16], F16)
                for cc in range(2):
                    wr = prep.tile([128, CO * 16], F32, tag="wdcr", name="wdcr")
                    nc.sync.dma_start(wr[:], wdc_in[cc * 128 : (cc + 1) * 128, :])
                    CP(wdc[:, cc, :], wr[:])

                def load_cvec(t_in, name):
                    t = wpool.tile([128, 2], F32, tag=name, name=name)
                    nc.sync.dma_start(t[:], dview(t_in.ap(), 0, [[1, 128], [128, 2]]))
                    return t

                g1 = load_cvec(g1_in, "g1")
                b1 = load_cvec(b1_in, "b1")
                g2 = load_cvec(g2_in, "g2")
                b2 = load_cvec(b2_in, "b2")

            # persistent buffers
            d_sb = big.tile([128, BPC, 2, P], F16)
            z_sb = big.tile([128, BPC, 2, 4, P], F16)
            bn1l = pipe.tile([128, 4], F32, tag="bn1l", name="bn1l")
            bn2l = pipe.tile([128, 4], F32, tag="bn2l", name="bn2l")
            nc.vector.memset(bn1l[:], 0.0)
            nc.vector.memset(bn2l[:], 0.0)

            ypad = big.tile([128, 2, PPIX], F16)
            nc.vector.memset(ypad[:, 0, :], 0.0)
            nc.vector.memset(ypad[:, 1, :], 0.0)

            # ============ per-batch prep: offset conv + index pipeline ======
            walls = []
            idxs = []
            with ex(name="psB", bufs=1, space="PSUM") as psB, ex(name="xp", bufs=1) as xpp:
                xpad = xpp.tile([128, BPC, 2, PPIX], F16)
                for b in range(BPC):
                    for cc in range(2):
                        nc.sync.dma_start(xpad[:, b, cc, :], xpad_in[b, cc])

                    # ---- offset conv ([18, P] channel-major) ----
                    ocp = psB.tile([18, P], F32, tag="ocp", name="ocp")
                    for half in range(2):
                        n0 = half * 512
                        first = True
                        for cc in range(2):
                            for k in range(9):
                                ky, kx = k // 3 - 1, k % 3 - 1
                                rhs = apv(
                                    xpad[:],
                                    b * 2 * PPIX + cc * PPIX
                                    + (1 + ky + half * 16) * PADW + 1 + kx,
                                    [[PADW, 16], [1, 32]],
                                )
                                nc.tensor.matmul(
                                    ocp[:, n0 : n0 + 512],
                                    wot[:, cc, k, :],
                                    rhs,
                                    start=first,
                                    stop=False,
                                )
                                first = False
                        nc.tensor.matmul(
                            ocp[:, n0 : n0 + 512], bof[:], ones512[:],
                            start=False, stop=True,
                        )
                    off_cm = pipe.tile([18, P], F32, tag="offcm", name="offcm")
                    CP(off_cm[:], ocp[:])
                    off_pm = pipe.tile([128, 8, 18], F32, tag="offpm", name="offpm")
                    for j in range(8):
                        tp = psB.tile([128, 18], F32, tag="tpo", name="tpo")
                        nc.tensor.transpose(
                            tp[:], off_cm[:, j * 128 : (j + 1) * 128], id32[0:18, 0:18]
                        )
                        CP(off_pm[:, j, :], tp[:])

                    # ---- index & weight pipeline ----
                    sh = [128, 8, 9]

                    def T(tag):
                        return pipe.tile(sh, F32, tag=tag, name=tag)

                    tmp_f = T("tmp_f")
                    tmp_g = T("tmp_g")
                    tmp_i = pipe.tile(sh, I32, tag="tmp_i", name="tmp_i")

                    def floor_(dst, src):
                        TS(tmp_f[:], src, 63.5, None, ALU.add)
                        CP(tmp_i[:], tmp_f[:])
                        CP(dst, tmp_i[:])
                        TS(dst, dst, -64.0, None, ALU.add)

                    def valid(dst, src, lo, hi):
                        TS(tmp_g[:], src, lo, None, ALU.is_ge)
                        TS(dst, src, hi, None, ALU.is_le)
                        TT(dst, dst, tmp_g[:], ALU.mult)

                    dy = apv(off_pm[:], 0, [[18, 8], [2, 9]])
                    dx = apv(off_pm[:], 1, [[18, 8], [2, 9]])
                    py, px = T("py"), T("px")
                    TT(py[:], hky[:], dy, ALU.add)
                    TT(px[:], wkx[:], dx, ALU.add)
                    y0, x0 = T("y0"), T("x0")
                    floor_(y0[:], py[:])
                    floor_(x0[:], px[:])
                    wy, wx = T("wy"), T("wx")
                    TT(wy[:], py[:], y0[:], ALU.subtract)
                    TT(wx[:], px[:], x0[:], ALU.subtract)
                    vy0, vy1 = T("vy0"), T("vy1")
                    valid(vy0[:], y0[:], 0.0, 31.0)
                    TS(tmp_f[:], y0[:], 1.0, None, ALU.add)
                    valid(vy1[:], tmp_f[:], 0.0, 31.0)
                    vx0, vx1 = T("vx0"), T("vx1")
                    valid(vx0[:], x0[:], 0.0, 31.0)
                    TS(tmp_f[:], x0[:], 1.0, None, ALU.add)
                    valid(vx1[:], tmp_f[:], 0.0, 31.0)

                    # x-slot selectors
                    xs = T("xs")
                    TS(xs[:], x0[:], 0.0, 30.0, ALU.max, ALU.min)
                    dl, dl2 = T("dl"), T("dl2")
                    TT(dl[:], x0[:], xs[:], ALU.subtract)
                    TT(dl2[:], dl[:], dl[:], ALU.mult)
                    i0, im, ip = T("i0"), T("im"), T("ip")
                    TS(i0[:], dl2[:], -1.0, 1.0, ALU.mult, ALU.add)
                    TT(im[:], dl2[:], dl[:], ALU.subtract)
                    TS(im[:], im[:], 0.5, None, ALU.mult)
                    TT(ip[:], dl2[:], dl[:], ALU.add)
                    TS(ip[:], ip[:], 0.5, None, ALU.mult)
                    w0, w1 = T("w0"), T("w1")
                    TS(tmp_f[:], wx[:], -1.0, 1.0, ALU.mult, ALU.add)
                    TT(w0[:], tmp_f[:], vx0[:], ALU.mult)
                    TT(w1[:], wx[:], vx1[:], ALU.mult)
                    ws0, ws1 = T("ws0"), T("ws1")
                    TT(ws0[:], w0[:], i0[:], ALU.mult)
                    TT(tmp_f[:], w1[:], im[:], ALU.mult)
                    TT(ws0[:], ws0[:], tmp_f[:], ALU.add)
                    TT(ws1[:], w1[:], i0[:], ALU.mult)
                    TT(tmp_f[:], w0[:], ip[:], ALU.mult)
                    TT(ws1[:], ws1[:], tmp_f[:], ALU.add)

                    # y-slot selectors (mirrored)
                    ysl = T("ysl")
                    TS(ysl[:], y0[:], 0.0, 30.0, ALU.max, ALU.min)
                    dly, dly2 = T("dly"), T("dly2")
                    TT(dly[:], y0[:], ysl[:], ALU.subtract)
                    TT(dly2[:], dly[:], dly[:], ALU.mult)
                    j0, jm, jp = T("j0"), T("jm"), T("jp")
                    TS(j0[:], dly2[:], -1.0, 1.0, ALU.mult, ALU.add)
                    TT(jm[:], dly2[:], dly[:], ALU.subtract)
                    TS(jm[:], jm[:], 0.5, None, ALU.mult)
                    TT(jp[:], dly2[:], dly[:], ALU.add)
                    TS(jp[:], jp[:], 0.5, None, ALU.mult)
                    a0, a1 = T("a0"), T("a1")
                    TS(tmp_f[:], wy[:], -1.0, 1.0, ALU.mult, ALU.add)
                    TT(a0[:], tmp_f[:], vy0[:], ALU.mult)
                    TT(a1[:], wy[:], vy1[:], ALU.mult)
                    as0, as1 = T("as0"), T("as1")
                    TT(as0[:], a0[:], j0[:], ALU.mult)
                    TT(tmp_f[:], a1[:], jm[:], ALU.mult)
                    TT(as0[:], as0[:], tmp_f[:], ALU.add)
                    TT(as1[:], a1[:], j0[:], ALU.mult)
                    TT(tmp_f[:], a0[:], jp[:], ALU.mult)
                    TT(as1[:], as1[:], tmp_f[:], ALU.add)

                    # wall: ci = s*2 + r  (s = x slot, r = y slot)
                    wall_pm = pipe.tile([128, 8, 36], F32, tag="wallpm", name="wallpm")
                    for ci, (rw, sl) in enumerate(
                        [(as0, ws0), (as1, ws0), (as0, ws1), (as1, ws1)]
                    ):
                        dst = apv(wall_pm[:], ci, [[36, 8], [4, 9]])
                        TT(dst, rw[:], sl[:], ALU.mult)
                    wall36 = pipe.tile([36, 8 * 128], F16, tag="wall36", name="wall36", bufs=2)
                    for j in range(8):
                        tpw = psB.tile([36, 128], F32, tag="tpw", name="tpw")
                        nc.tensor.transpose(tpw[:], wall_pm[:, j, :], id32[:])
                        CP(wall36[:, j * 128 : (j + 1) * 128], tpw[:])
                    walls.append(wall36)

                    # token index: tok = ys*32 + xs + b*1024
                    tok = T("tok")
                    TS(tmp_f[:], ysl[:], 32.0, float(b * P), ALU.mult, ALU.add)
                    TT(tok[:], tmp_f[:], xs[:], ALU.add)

                    idx128 = pipe.tile([128, 9, 64], I16, tag="idx", name="idx", bufs=2)
                    pf = psB.tile([16, 1024], F32, tag="pf", name="pf")
                    for a in range(8):
                        nc.tensor.matmul(
                            pf[:, a * 128 : a * 128 + 72],
                            id32[:, a * 16 : (a + 1) * 16],
                            tok[:].rearrange("p a b -> p (a b)"),
                            start=True,
                            stop=True,
                        )
                    src = apv(pf[:], 0, [[1, 9], [9, 8], [128, 8]])
                    dst = apv(idx128[:], 0, [[64, 9], [8, 8], [1, 8]], nparts=16)
                    CP(dst, src)
                    for lo, hi in ((16, 32), (32, 64), (64, 128)):
                        nc.sync.dma_start(
                            idx128[lo:hi, :, :], idx128[0 : hi - lo, :, :]
                        )
                    idxs.append(idx128)

            # ============ tap loops: gather, weights, combine, matmuls ======
            src_ap = dview(xt2_in, 0, [[512, BPC * P], [1, 1024]])
            with (
                ex(name="gt", bufs=2) as gtp,
                ex(name="st", bufs=2) as stp,
                ex(name="rp", bufs=4, space="PSUM") as rpp,
                ex(name="dc", bufs=1, space="PSUM") as dcp,
            ):
                for b in range(BPC):
                    wall36 = walls[b]
                    idx128 = idxs[b]
                    dacc = [
                        [dcp.tile([128, 512], F32, tag=f"dacc{co}{hf}", name=f"dacc{co}{hf}") for hf in range(2)]
                        for co in range(2)
                    ]
                    for k in range(9):
                        gt = gtp.tile([128, 8, P], F16, tag="gt", name="gt")
                        nc.gpsimd.dma_gather(
                            gt[:],
                            src_ap,
                            idx128[:, k, :],
                            num_idxs=P,
                            num_idxs_reg=P,
                            elem_size=1024,
                            elem_step=512,
                            transpose=True,
                            single_packet=False,
                        )
                        # replicate combine weights: rr_s[:, r, :] = wall row k*4+s*2+r
                        rr = []
                        for s in range(2):
                            rt = stp.tile([128, 2, P], F16, tag=f"rt{s}", name=f"rt{s}")
                            for r in range(2):
                                krs = k * 4 + s * 2 + r
                                for hf in range(2):
                                    rps = rpp.tile([128, 512], F32, tag="rph", name="rph")
                                    nc.tensor.matmul(
                                        rps[:],
                                        e36[:, krs * 128 : (krs + 1) * 128],
                                        wall36[:, hf * 512 : (hf + 1) * 512],
                                        start=True,
                                        stop=True,
                                    )
                                    nc.scalar.copy(
                                        rt[:, r, hf * 512 : (hf + 1) * 512], rps[:]
                                    )
                            rr.append(rt)
                        for cc in range(2):
                            # data: e = s*4 + r*2 + cc ; (r, i) pairs stride 2048
                            p1 = stp.tile([128, 2, P], F16, tag="p1", name="p1", bufs=1)
                            p2 = stp.tile([128, 2, P], F16, tag="p2", name="p2", bufs=1)
                            st = stp.tile([128, 2, P], F16, tag=f"st{cc}", name=f"st{cc}")
                            gL = apv(gt[:], (0 * 4 + cc) * P, [[2 * P, 2], [1, P]])
                            gR = apv(gt[:], (1 * 4 + cc) * P, [[2 * P, 2], [1, P]])
                            TT(p1[:], gL, rr[0][:], ALU.mult)
                            TT(p2[:], gR, rr[1][:], ALU.mult)
                            if GP_ADD:
                                nc.gpsimd.scalar_tensor_tensor(
                                    st[:], p1[:], 1.0, p2[:], ALU.mult, ALU.add
                                )
                            else:
                                TT(st[:], p1[:], p2[:], ALU.add)
                            for co in range(2):
                                for r in range(2):
                                    for hf in range(2):
                                        nc.tensor.matmul(
                                            dacc[co][hf][:],
                                            wdt[:, k, cc, co, :],
                                            st[:, r, hf * 512 : (hf + 1) * 512],
                                            start=(k == 0 and cc == 0 and r == 0),
                                            stop=(k == 8 and cc == 1 and r == 1),
                                        )
                    for co in range(2):
                        for hf in range(2):
                            if hf == 0:
                                CP(d_sb[:, b, co, hf * 512 : (hf + 1) * 512], dacc[co][hf][:])
                            else:
                                nc.scalar.copy(
                                    d_sb[:, b, co, hf * 512 : (hf + 1) * 512], dacc[co][hf][:]
                                )
                        part = pipe.tile([128, 1], F32, tag="part", name="part")
                        sq16 = wk.tile([128, P], F16, tag="sq16", name="sq16")
                        nc.scalar.activation(
                            sq16[:], d_sb[:, b, co, :], ACTF.Copy, accum_out=part[:]
                        )
                        TT(bn1l[:, co : co + 1], bn1l[:, co : co + 1], part[:], ALU.add)
                        nc.scalar.activation(
                            sq16[:], d_sb[:, b, co, :], ACTF.Square, accum_out=part[:]
                        )
                        TT(
                            bn1l[:, 2 + co : 3 + co],
                            bn1l[:, 2 + co : 3 + co],
                            part[:],
                            ALU.add,
                        )

            # ================= BN1 allreduce + coeffs =================
            def allreduce_stats(local_tile, tag):
                src_d = dpool.tile([128, 4], F32, tag=f"ari_{tag}", name=f"ari_{tag}")
                dst_d = dpool.tile([128, 4], F32, tag=f"aro_{tag}", name=f"aro_{tag}")
                nc.gpsimd.dma_start(src_d, local_tile[:])
                nc.gpsimd.collective_compute(
                    "AllReduce",
                    ALU.add,
                    replica_groups=groups or [list(range(NCORES))],
                    ins=[src_d.opt()],
                    outs=[dst_d.opt()],
                )
                g = pipe.tile([128, 4], F32, tag=f"ars_{tag}", name=f"ars_{tag}")
                nc.gpsimd.dma_start(g[:], dst_d)
                return g

            def bn_coeffs(stats, gam, bet, count, tag):
                sc = pipe.tile([128, 2], F32, tag=f"sc_{tag}", name=f"sc_{tag}")
                bi = pipe.tile([128, 2], F32, tag=f"bi_{tag}", name=f"bi_{tag}")
                mean = pipe.tile([128, 2], F32, tag=f"mean_{tag}", name=f"mean_{tag}")
                var = pipe.tile([128, 2], F32, tag=f"var_{tag}", name=f"var_{tag}")
                t2 = pipe.tile([128, 2], F32, tag=f"t2_{tag}", name=f"t2_{tag}")
                TS(mean[:], stats[:, 0:2], 1.0 / count, None, ALU.mult)
                TS(var[:], stats[:, 2:4], 1.0 / count, None, ALU.mult)
                TT(t2[:], mean[:], mean[:], ALU.mult)
                TT(var[:], var[:], t2[:], ALU.subtract)
                TS(var[:], var[:], EPS, None, ALU.add)
                nc.scalar.activation(var[:], var[:], ACTF.Sqrt)
                nc.vector.reciprocal(var[:], var[:])
                TT(sc[:], gam[:], var[:], ALU.mult)
                TT(t2[:], mean[:], sc[:], ALU.mult)
                TT(bi[:], bet[:], t2[:], ALU.subtract)
                return sc, bi

            bn1g = allreduce_stats(bn1l, "bn1")
            sc1, bi1 = bn_coeffs(bn1g, g1, b1, bn_b * P, "bn1")

            # ================= convT phase =================
            TAPS = {0: [(1, 0), (3, -1)], 1: [(0, 1), (2, 0)]}
            with ex(name="zp", bufs=4, space="PSUM") as zpp:
                for b in range(BPC):
                    for cc in range(2):
                        dst = apv(ypad[:], cc * PPIX + PADW + 1, [[PADW, 32], [1, 32]])
                        nc.scalar.activation(
                            dst,
                            d_sb[:, b, cc, :].rearrange("p (h w) -> p h w", h=32),
                            ACTF.Silu,
                            bias=bi1[:, cc : cc + 1],
                            scale=sc1[:, cc : cc + 1],
                        )
                    for ph in range(4):
                        ry, rx = ph // 2, ph % 2
                        for co in range(2):
                            for hf in range(2):
                                zp = zpp.tile([128, 512], F32, tag="zp", name="zp")
                                first = True
                                for (kyy, dyy) in TAPS[ry]:
                                    for (kxx, dxx) in TAPS[rx]:
                                        for cc in range(2):
                                            rhs = apv(
                                                ypad[:],
                                                cc * PPIX
                                                + (1 + dyy + hf * 16) * PADW
                                                + 1
                                                + dxx,
                                                [[PADW, 16], [1, 32]],
                                            )
                                            lhsT = apv(
                                                wdc[:],
                                                cc * (CO * 16)
                                                + co * 2048
                                                + kyy * 4
                                                + kxx,
                                                [[16, 128]],
                                            )
                                            nc.tensor.matmul(
                                                zp[:],
                                                lhsT,
                                                rhs,
                                                start=first,
                                                stop=(
                                                    kyy == TAPS[ry][1][0]
                                                    and kxx == TAPS[rx][1][0]
                                                    and cc == 1
                                                ),
                                            )
                                            first = False
                                if (ph + co) % 2 == 0:
                                    CP(
                                        z_sb[:, b, co, ph, hf * 512 : (hf + 1) * 512],
                                        zp[:],
                                    )
                                else:
                                    nc.scalar.copy(
                                        z_sb[:, b, co, ph, hf * 512 : (hf + 1) * 512],
                                        zp[:],
                                    )
                    for co in range(2):
                        part = pipe.tile([128, 1], F32, tag="part2", name="part2")
                        sq16z = wk.tile([128, 4 * P], F16, tag="sq16z", name="sq16z")
                        nc.scalar.activation(
                            sq16z[:], z_sb[:, b, co, :, :], ACTF.Copy, accum_out=part[:]
                        )
                        TT(bn2l[:, co : co + 1], bn2l[:, co : co + 1], part[:], ALU.add)
                        nc.scalar.activation(
                            sq16z[:], z_sb[:, b, co, :, :], ACTF.Square, accum_out=part[:]
                        )
                        TT(
                            bn2l[:, 2 + co : 3 + co],
                            bn2l[:, 2 + co : 3 + co],
                            part[:],
                            ALU.add,
                        )

            bn2g = allreduce_stats(bn2l, "bn2")
            sc2, bi2 = bn_coeffs(bn2g, g2, b2, bn_b * PO, "bn2")

            # ================= final BN2+SiLU + output =================
            with ex(name="outst", bufs=2) as outp:
                for b in range(BPC):
                    for co in range(2):
                        ost = outp.tile([128, PO], F16, tag="ost", name="ost")
                        for ph in range(4):
                            ry, rx = ph // 2, ph % 2
                            dst = apv(ost[:], ry * 64 + rx, [[128, 32], [2, 32]])
                            nc.scalar.activation(
                                dst,
                                z_sb[:, b, co, ph, :].rearrange("p (h w) -> p h w", h=32),
                                ACTF.Silu,
                                bias=bi2[:, co : co + 1],
                                scale=sc2[:, co : co + 1],
                            )
                        nc.sync.dma_start(out_t[b, co * 128 : (co + 1) * 128, :], ost[:])

    nc.finalize()
    return nc


_NC_CACHE = {}


def make_in_maps(inputs):
    x = np.ascontiguousarray(inputs["x"], dtype=np.float32)
    w_off = np.ascontiguousarray(inputs["w_off"], dtype=np.float32).reshape(18, C * 9)
    b_off = np.ascontiguousarray(inputs["b_off"], dtype=np.float32)
    w_def = np.ascontiguousarray(inputs["w_def"], dtype=np.float32).reshape(CO, C * 9)
    w_dc = np.ascontiguousarray(inputs["w_dc"], dtype=np.float32).reshape(C, CO * 16)
    g1 = np.ascontiguousarray(inputs["gamma1"], dtype=np.float32)
    b1 = np.ascontiguousarray(inputs["beta1"], dtype=np.float32)
    g2 = np.ascontiguousarray(inputs["gamma2"], dtype=np.float32)
    b2 = np.ascontiguousarray(inputs["beta2"], dtype=np.float32)

    x16 = x.astype(np.float16)
    # xt2[b, y, x] = [row y, row y+1] channels (row-pair duplicated, token-major)
    xt = x16.transpose(0, 2, 3, 1).reshape(B, H, W, C)
    xt2 = np.zeros((B, H, W, 2, C), np.float16)
    xt2[:, :, :, 0, :] = xt
    xt2[:, : H - 1, :, 1, :] = xt[:, 1:, :, :]
    xt2 = xt2.reshape(B, P, 512)
    # xpad: zero-padded 34x34 channel-major
    xpad = np.zeros((B, 2, 128, PADW, PADW), np.float16)
    xpad[:, :, :, 1:33, 1:33] = x16.reshape(B, 2, 128, H, W)
    xpad = xpad.reshape(B, 2, 128, PPIX)

    in_maps = []
    for core in range(NCORES):
        sl = slice(core * BPC, (core + 1) * BPC)
        in_maps.append(
            {
                "xt2": np.concatenate(
                    [xt2[sl].reshape(BPC * P, 512), np.zeros((1, 512), np.float16)]
                ),
                "xpad": np.ascontiguousarray(xpad[sl]),
                "w_off": w_off,
                "b_off": b_off,
                "w_def": w_def,
                "w_dc": w_dc,
                "gamma1": g1,
                "beta1": b1,
                "gamma2": g2,
                "beta2": b2,
            }
        )
    return in_maps


def kernel(**inputs):
    if "nc" not in _NC_CACHE:
        _NC_CACHE["nc"] = build_program()
    nc = _NC_CACHE["nc"]
    in_maps = make_in_maps(inputs)
    res = run_bass_kernel_spmd(nc, in_maps, list(range(NCORES)))
    out = np.concatenate([res.results[i]["out"] for i in range(NCORES)], axis=0)
    return out.reshape(B, CO, HO, WO).astype(np.float32)
